# revision 45
# baseline (speedup 1.0000x reference)
"""Trainium2 Bass kernel for the nn_Dynamics problem (v2).

Math (per batch element, d=8, H=128), with g' = -g sign convention:
  z0 = W0 x + b0; h0 = tanh(z0); t0 = W0 v
  h0p' = (h0^2-1) t0        [= -dh0/deps]
  z1 = W1 h0 + b1; h1 = tanh(z1); t1' = W1 h0p'   [= -t1]
  q1 = h1^2;  A0 = c_A0 - W1w^T q1  with W1w = diag(w2) W1, c_A0 = colsum(W1w)
  a0' = (h0^2-1) A0 = -a0;  g' = W0^T a0' = -g
  e1 = h1 (1-h1^2) t1^2;  w0 = h0 t0^2;  e2 = a0' w0 = A0 u'
  hvv = -2 sum(w2*e1) + 2 sum(e2)
  force = -(K x + D v)                               [computed on HOST]
  out = force + g' * (hvv - g'.force) / (1 + |g'|^2)   (Sherman-Morrison)

Device layout: feature-major [128 features, batch] tiles of TW=512 for the
MLP/backward; the per-batch reductions (g, hvv) are computed batch-major by
flipped matmuls (data chunk as the stationary operand, small weights moving),
so no on-chip transposes at all: X arrives pre-transposed from the host, the
output leaves in a device-friendly packed layout and is unpermuted on host.

The loop is software-pipelined across tiles (stages j, j-1, j-2, j-3) so each
engine queue (PE / ACT / DVE) stays dense and cross-engine deps never block
an engine's queue head for long.

Sharding: pure data parallel over 8 NeuronCores (8192 rows each), weights
replicated, outputs concatenated (host re-permutes per-core blocks).
"""

import os
from contextlib import ExitStack

import numpy as np

import concourse.bacc as bacc
import concourse.bass as bass
import concourse.dve_ops as dve_ops
import concourse.tile as tile
from concourse import mybir
from concourse.bass_utils import run_bass_kernel_spmd
from concourse.dve_ops import DveOp
from concourse.dve_ops import has_src1
from concourse.dve_spec import C0, One, Spec, Src0, Src1, lower, sq
from concourse.dve_uop import DveOpSpec

F32 = mybir.dt.float32
F16 = mybir.dt.float16
AX = mybir.AxisListType
OP = mybir.AluOpType
ACT = mybir.ActivationFunctionType

DIM = 8
H = 128
BATCH = 65536
NCORES = 8
BC = BATCH // NCORES          # 8192 rows per core
TW = 512                      # batch tile width
NT = BC // TW                 # 16 tiles per core
NCH = TW // 128               # 4 chunks of 128 per tile
GT = 8                        # tiles per tail group
CW = DIM + 1                  # bm cols per chunk: g'(8) + hvv(1)

LAST_RESULTS = None

# ---------------- custom fused DVE ops ----------------


def _register_op(name, body, reference):
    if name in dve_ops._SUB_OPCODE_FOR_NAME:
        for op in dve_ops.OPS:
            if op.name == name:
                return op
    spec = Spec(body=body, reference=reference)
    shas = {}
    for ver in ("v3", "v4"):
        shas[ver] = DveOpSpec(
            name=name,
            opcode=dve_ops._CUSTOM_DVE_ROW_BASE + len(dve_ops.OPS),
            uops=lower(spec, ver=ver),
            rd1_en=has_src1(spec),
        ).sha(ver)
    op = DveOp(name, spec, subdim=False, uops_sha=shas)
    dve_ops.OPS.append(op)
    dve_ops.CUSTOM_DVE_SPECS[name] = spec
    dve_ops._SUB_OPCODE_FOR_NAME[name] = (
        dve_ops._CUSTOM_DVE_ROW_BASE + len(dve_ops.OPS) - 1
    )
    return op


# h0p' = (h0^2 - 1) * t0
OP_SQM1_MUL = _register_op(
    "ANT_SQM1_MUL",
    (sq(Src0) - One) * Src1,
    lambda in0, in1: (in0 * in0 - 1.0) * in1,
)
# a0' = (h0^2 - 1) * (c_A0 - A0raw)
OP_A0G = _register_op(
    "ANT_A0G",
    (sq(Src0) - One) * (C0 - Src1),
    lambda in0, in1, s0: (in0 * in0 - 1.0) * (s0 - in1),
)
# e1 = h1 * (1 - h1^2) * t1^2
OP_E1G = _register_op(
    "ANT_E1G",
    Src0 * (One - sq(Src0)) * sq(Src1),
    lambda in0, in1: in0 * (1.0 - in0 * in0) * in1 * in1,
)


def build_nc():
    nc = bacc.Bacc()

    XTx = nc.dram_tensor("XTx", [DIM, BC], F16, kind="ExternalInput")
    XTv = nc.dram_tensor("XTv", [DIM, BC], F16, kind="ExternalInput")
    Fh = nc.dram_tensor("Fh", [128, NT * NCH * DIM], F16, kind="ExternalInput")
    # all small weights in two packed images (one DMA each)
    # pack16 cols: W1T[0:128] W1w[128:256] W08-block[256:384] W0g[384:392]
    #              w2m2[392] p2[393]
    Wp16 = nc.dram_tensor("Wp16", [128, 394], F16, kind="ExternalInput")
    Wp32 = nc.dram_tensor("Wp32", [128, 3], F32, kind="ExternalInput")
    out = nc.dram_tensor("out", [128, NT * NCH * DIM], F32, kind="ExternalOutput")

    with tile.TileContext(nc) as tc, ExitStack() as stk:
        consts = stk.enter_context(tc.tile_pool(name="consts", bufs=1))
        sb = stk.enter_context(tc.tile_pool(name="sb", bufs=2))
        ps_z0 = stk.enter_context(tc.tile_pool(name="psz0", bufs=2, space="PSUM"))
        ps_t0 = stk.enter_context(tc.tile_pool(name="pst0", bufs=2, space="PSUM"))
        ps_zA = stk.enter_context(tc.tile_pool(name="pszA", bufs=2, space="PSUM"))
        ps_t1 = stk.enter_context(tc.tile_pool(name="pst1", bufs=1, space="PSUM"))
        ps_bm = stk.enter_context(tc.tile_pool(name="psbm", bufs=1, space="PSUM"))

        # ---------------- constants / inputs ----------------
        XT64 = consts.tile([40, BC], F16)
        wp16 = consts.tile([128, 394], F16)
        wp32 = consts.tile([128, 3], F32)
        force_sb = consts.tile([128, NT * NCH * DIM], F16)
        # X transposed halves arrive in 4 column chunks so tile 0's matmuls
        # unblock after the first quarter of the transfer (subtile deps);
        # XTv goes through the scalar engine's queue so the two streams'
        # issue+transfer overlap.
        XQ = BC // 4
        nc.sync.dma_start(out=XT64[0:DIM, 0:XQ], in_=XTx[:, 0:XQ])
        nc.scalar.dma_start(out=XT64[32:40, 0:XQ], in_=XTv[:, 0:XQ])
        nc.sync.dma_start(out=wp16, in_=Wp16[:, :])
        nc.sync.dma_start(out=wp32, in_=Wp32[:, :])
        for q in range(1, 4):
            qs = slice(q * XQ, (q + 1) * XQ)
            nc.sync.dma_start(out=XT64[0:DIM, qs], in_=XTx[:, qs])
            nc.scalar.dma_start(out=XT64[32:40, qs], in_=XTv[:, qs])
        nc.gpsimd.dma_start(out=force_sb, in_=Fh[:, :])
        W1T_sb = wp16[:, 0:128]
        W1w_sb = wp16[:, 128:256]
        W08_sb = wp16[0:40, 256:384]
        W0g_sb = wp16[:, 384:392]
        w2m2_sb = wp16[:, 392:393]
        p2_sb = wp16[:, 393:394]
        b0_sb = wp32[:, 0:1]
        b1_sb = wp32[:, 1:2]
        cA0_sb = wp32[:, 2:3]

        out_sb = consts.tile([128, NT * NCH * DIM], F32)

        # HAM warmup: ~14 back-to-back dummy matmuls keep the PE busy for a
        # full activity window so the clock gate opens (1.2 -> 2.4 GHz) while
        # the input DMAs land; the main loop's PE gaps are all far below the
        # re-throttle window, so the array stays at full clock afterwards.
        wg = consts.tile([128, H], F16)
        wr = consts.tile([128, TW], F16)
        nc.vector.memset(wg, 0.0)
        nc.vector.memset(wr, 0.0)
        wps = ps_z0.tile([128, TW], F32, tag="z0")
        for _ in range(6):
            nc.tensor.matmul(wps, wg, wr, start=True, stop=True)

        st = [dict() for _ in range(NT)]
        bm_cur = [None]

        def ap3(t, off, dims):
            return bass.AP(
                tensor=t.tensor, offset=t.offset + off, ap=[list(t.ap[0])] + dims
            )

        # ---------------- software-pipelined main loop ----------------
        for i in range(NT + 3):
            j0, j1, j2, j3 = i, i - 1, i - 2, i - 3

            if 0 <= j0 < NT:
                s = st[j0]
                cols = slice(j0 * TW, (j0 + 1) * TW)
                z0 = ps_z0.tile([128, TW], F32, tag="z0")
                nc.tensor.matmul(
                    z0, W08_sb[0:DIM, :], XT64[0:DIM, cols], start=True, stop=True
                )
                t0 = ps_t0.tile([128, TW], F32, tag="t0")
                nc.tensor.matmul(
                    t0, W08_sb[32:40, :], XT64[32:40, cols], start=True, stop=True
                )
                s["z0"], s["t0"] = z0, t0

            if 0 <= j2 < NT:
                # e1's inputs (h1, t1 of tile j2) completed last iteration, so
                # it leads the DVE queue and fills the wait for this
                # iteration's h0 activation.
                s = st[j2]
                e1 = sb.tile([128, TW], F16, tag="e1")
                nc.vector._custom_dve(OP_E1G, out=e1, in0=s["h1"], in1=s["t1"][:, :])
                s["e1"] = e1
                A0 = ps_zA.tile([128, TW], F32, tag="zA")
                nc.tensor.matmul(A0, W1w_sb, s["q1"], start=True, stop=True)
                s["A0"] = A0

            if 0 <= j1 < NT:
                s = st[j1]
                h0 = sb.tile([128, TW], F16, tag="h0")
                nc.scalar.activation(h0, s["z0"], ACT.Tanh, bias=b0_sb[:, 0:1])
                t0sq = sb.tile([128, TW], F16, tag="t0sq")
                nc.scalar.activation(t0sq, s["t0"], ACT.Square)
                h0p = sb.tile([128, TW], F16, tag="h0p")
                nc.vector._custom_dve(OP_SQM1_MUL, out=h0p, in0=h0, in1=s["t0"][:, :])
                w0 = sb.tile([128, TW], F16, tag="w0")
                nc.gpsimd.tensor_mul(w0, h0, t0sq)
                z1 = ps_zA.tile([128, TW], F32, tag="zA")
                nc.tensor.matmul(z1, W1T_sb, h0, start=True, stop=True)
                h1 = sb.tile([128, TW], F16, tag="h1")
                nc.scalar.activation(h1, z1, ACT.Tanh, bias=b1_sb[:, 0:1])
                q1 = sb.tile([128, TW], F16, tag="q1")
                nc.scalar.activation(q1, h1, ACT.Square)
                s["h0"], s["h0p"], s["w0"], s["h1"], s["q1"] = h0, h0p, w0, h1, q1

            if 0 <= j2 < NT:
                s = st[j2]
                a0p = sb.tile([128, TW], F16, tag="a0p")
                nc.vector._custom_dve(
                    OP_A0G, out=a0p, in0=s["h0"], in1=s["A0"][:, :], s0=cA0_sb[:, 0:1]
                )
                e2 = sb.tile([128, TW], F16, tag="e2")
                nc.vector.tensor_mul(e2, a0p, s["w0"])
                s["a0p"], s["e2"] = a0p, e2

            if 0 <= j1 < NT:
                s = st[j1]
                t1 = ps_t1.tile([128, TW], F32, tag="t1")
                nc.tensor.matmul(t1, W1T_sb, s["h0p"], start=True, stop=True)
                s["t1"] = t1

            if 0 <= j3 < NT:
                # flipped per-chunk reductions for tile j3 (inputs from i-1);
                # late in the PE queue: nothing reads bm until the group's
                # bmc copy, so only z1/A0 need to run early for the DVE chain.
                s = st[j3]
                if j3 % GT == 0:
                    bm_cur[0] = ps_bm.tile(
                        [128, GT * NCH * CW], F32, tag="bm", name="bm"
                    )
                bm = bm_cur[0]
                base = (j3 % GT) * NCH * CW
                for c in range(NCH):
                    csl = slice(c * 128, (c + 1) * 128)
                    off = base + c * CW
                    nc.tensor.matmul(
                        bm[:, off : off + DIM],
                        s["a0p"][:, csl],
                        W0g_sb,
                        start=True,
                        stop=True,
                    )
                    nc.tensor.matmul(
                        bm[:, off + DIM : off + CW],
                        s["e1"][:, csl],
                        w2m2_sb,
                        start=True,
                        stop=False,
                    )
                    nc.tensor.matmul(
                        bm[:, off + DIM : off + CW],
                        s["e2"][:, csl],
                        p2_sb,
                        start=False,
                        stop=True,
                    )

            if 0 <= j3 < NT:
                bm = bm_cur[0]
                g = j3 // GT
                if j3 % GT == GT - 1:
                    # -------- batch-major tail for group g (GT*NCH chunks) --------
                    NCK = GT * NCH  # 16 chunks of 128 batch rows
                    bmc = sb.tile([128, NCK * CW], F32, tag="bmc")
                    nc.scalar.copy(bmc, bm[:, 0 : NCK * CW])
                    gv = ap3(bmc, 0, [[CW, NCK], [1, DIM]])
                    hv = ap3(bmc, DIM, [[CW, NCK]])
                    fv = ap3(force_sb, g * NCK * DIM, [[DIM, NCK], [1, DIM]])
                    prod = sb.tile([128, 2 * NCK * DIM], F32, tag="prod")
                    prod4 = prod.rearrange("p (q k f) -> p q k f", q=2, f=DIM)
                    nc.vector.tensor_mul(prod4[:, 0, :, :], gv, gv)
                    nc.vector.tensor_mul(prod4[:, 1, :, :], gv, fv)
                    red = sb.tile([128, 2 * NCK], F32, tag="red")
                    red3 = red.rearrange("p (q k) -> p q k", q=2)
                    nc.vector.tensor_reduce(red3, prod4, axis=AX.X, op=OP.add)
                    den = sb.tile([128, NCK], F32, tag="den")
                    nc.vector.tensor_scalar_add(den, red3[:, 0, :], 1.0)
                    rec = sb.tile([128, NCK], F32, tag="rec")
                    nc.vector.reciprocal(rec, den)
                    num = sb.tile([128, NCK], F32, tag="num")
                    nc.vector.tensor_sub(num, hv, red3[:, 1, :])
                    s2 = sb.tile([128, NCK], F32, tag="s2")
                    nc.vector.tensor_mul(s2, num, rec)
                    s2b = bass.AP(
                        tensor=s2.tensor,
                        offset=s2.offset,
                        ap=[list(s2.ap[0]), [1, NCK], [0, DIM]],
                    )
                    su = sb.tile([128, NCK * DIM], F32, tag="su")
                    su3 = su.rearrange("p (k f) -> p k f", f=DIM)
                    nc.vector.tensor_mul(su3, gv, s2b)
                    ov = ap3(out_sb, g * NCK * DIM, [[DIM, NCK], [1, DIM]])
                    nc.vector.tensor_add(ov, su3, fv)

        nc.sync.dma_start(out=out[:, :], in_=out_sb)

    if not nc.is_finalized():
        nc.finalize()

    return nc


_NC_CACHE = None


def _install_ntff_shim():
    """Register the axon NTFF profile hook (missing antenv.axon_hooks shim)."""
    import sys
    import types

    if "antenv.axon_hooks" in sys.modules:
        return
    try:
        sys.path.insert(0, "/root/.axon_site")
        from trn_agent_boot.trn_boot import _ntff_profile_via_ctypes

        hook = _ntff_profile_via_ctypes("/opt/axon/libaxon_pjrt.so")
        mod = types.ModuleType("antenv.axon_hooks")
        mod.get_axon_ntff_profile_hook = lambda: hook
        sys.modules["antenv.axon_hooks"] = mod
    except Exception:
        pass


def kernel(**inputs):
    global LAST_RESULTS, _NC_CACHE
    trace = bool(int(os.environ.get("KERNEL_TRACE", "0")))
    if trace:
        _install_ntff_shim()
    if _NC_CACHE is None:
        _NC_CACHE = build_nc()
    nc = _NC_CACHE

    f16 = np.float16
    X = np.asarray(inputs["X"], np.float32)
    K = np.asarray(inputs["K"], np.float32)
    D = np.asarray(inputs["D"], np.float32)
    W0 = np.asarray(inputs["W0"], np.float32)
    W1 = np.asarray(inputs["W1"], np.float32)
    w2 = np.asarray(inputs["W2"], np.float32).reshape(-1)

    w1w16 = (w2[:, None] * W1).astype(f16)
    ca0 = w1w16.astype(np.float32).sum(axis=0).astype(np.float32)

    wp16 = np.zeros((128, 394), f16)
    wp16[:, 0:128] = W1.T.astype(f16)
    wp16[0:DIM, 256:384] = W0.T.astype(f16)
    wp16[32:40, 256:384] = W0.T.astype(f16)
    wp16[:, 128:256] = w1w16
    wp16[:, 384:392] = W0.astype(f16)
    wp16[:, 392] = (-2.0 * w2).astype(f16)
    wp16[:, 393] = f16(2.0)
    wp32 = np.zeros((128, 3), np.float32)
    wp32[:, 0] = np.asarray(inputs["b0"], np.float32)
    wp32[:, 1] = np.asarray(inputs["b1"], np.float32)
    wp32[:, 2] = ca0

    KD = np.concatenate([K, D], axis=1)          # [8, 16]
    force = -(X @ KD.T)                           # [B, 8] f32 on host

    shared = {"Wp16": wp16, "Wp32": wp32}

    in_maps = []
    for i in range(NCORES):
        Xi = X[i * BC : (i + 1) * BC]
        Fi = force[i * BC : (i + 1) * BC]
        m = {
            "XTx": np.ascontiguousarray(Xi[:, :DIM].T).astype(f16),
            "XTv": np.ascontiguousarray(Xi[:, DIM:].T).astype(f16),
            "Fh": np.ascontiguousarray(
                Fi.reshape(NT, NCH, 128, DIM).transpose(2, 0, 1, 3).reshape(128, -1)
            ).astype(f16),
        }
        m.update(shared)
        in_maps.append(m)

    res = run_bass_kernel_spmd(
        nc, in_maps, core_ids=list(range(NCORES)), trace=trace
    )
    LAST_RESULTS = res
    outs = []
    for i in range(NCORES):
        o = res.results[i]["out"]  # [128, NT*NCH*DIM]
        o = o.reshape(128, NT, NCH, DIM).transpose(1, 2, 0, 3).reshape(BC, DIM)
        outs.append(o)
    return np.concatenate(outs, axis=0).astype(np.float32)


# revision 47
# speedup vs baseline: 1.0407x; 1.0407x over previous
"""Trainium2 Bass kernel for the nn_Dynamics problem (v2).

Math (per batch element, d=8, H=128), with g' = -g sign convention:
  z0 = W0 x + b0; h0 = tanh(z0); t0 = W0 v
  h0p' = (h0^2-1) t0        [= -dh0/deps]
  z1 = W1 h0 + b1; h1 = tanh(z1); t1' = W1 h0p'   [= -t1]
  q1 = h1^2;  A0 = c_A0 - W1w^T q1  with W1w = diag(w2) W1, c_A0 = colsum(W1w)
  a0' = (h0^2-1) A0 = -a0;  g' = W0^T a0' = -g
  e1 = h1 (1-h1^2) t1^2;  w0 = h0 t0^2;  e2 = a0' w0 = A0 u'
  hvv = -2 sum(w2*e1) + 2 sum(e2)
  force = -(K x + D v)                               [computed on HOST]
  out = force + g' * (hvv - g'.force) / (1 + |g'|^2)   (Sherman-Morrison)

Device layout: feature-major [128 features, batch] tiles of TW=512 for the
MLP/backward; the per-batch reductions (g, hvv) are computed batch-major by
flipped matmuls (data chunk as the stationary operand, small weights moving),
so no on-chip transposes at all: X arrives pre-transposed from the host, the
output leaves in a device-friendly packed layout and is unpermuted on host.

The loop is software-pipelined across tiles (stages j, j-1, j-2, j-3) so each
engine queue (PE / ACT / DVE) stays dense and cross-engine deps never block
an engine's queue head for long.

Sharding: pure data parallel over 8 NeuronCores (8192 rows each), weights
replicated, outputs concatenated (host re-permutes per-core blocks).
"""

import os
from contextlib import ExitStack

import numpy as np

import concourse.bacc as bacc
import concourse.bass as bass
import concourse.dve_ops as dve_ops
import concourse.tile as tile
from concourse import mybir
from concourse.bass_utils import run_bass_kernel_spmd
from concourse.dve_ops import DveOp
from concourse.dve_ops import has_src1
from concourse.dve_spec import C0, One, Spec, Src0, Src1, lower, sq
from concourse.dve_uop import DveOpSpec

F32 = mybir.dt.float32
F16 = mybir.dt.float16
AX = mybir.AxisListType
OP = mybir.AluOpType
ACT = mybir.ActivationFunctionType

DIM = 8
H = 128
BATCH = 65536
NCORES = 8
BC = BATCH // NCORES          # 8192 rows per core
TW = 512                      # batch tile width
NT = BC // TW                 # 16 tiles per core
NCH = TW // 128               # 4 chunks of 128 per tile
GT = 8                        # tiles per tail group
CW = DIM + 1                  # bm cols per chunk: g'(8) + hvv(1)

LAST_RESULTS = None

# ---------------- custom fused DVE ops ----------------


def _register_op(name, body, reference):
    if name in dve_ops._SUB_OPCODE_FOR_NAME:
        for op in dve_ops.OPS:
            if op.name == name:
                return op
    spec = Spec(body=body, reference=reference)
    shas = {}
    for ver in ("v3", "v4"):
        shas[ver] = DveOpSpec(
            name=name,
            opcode=dve_ops._CUSTOM_DVE_ROW_BASE + len(dve_ops.OPS),
            uops=lower(spec, ver=ver),
            rd1_en=has_src1(spec),
        ).sha(ver)
    op = DveOp(name, spec, subdim=False, uops_sha=shas)
    dve_ops.OPS.append(op)
    dve_ops.CUSTOM_DVE_SPECS[name] = spec
    dve_ops._SUB_OPCODE_FOR_NAME[name] = (
        dve_ops._CUSTOM_DVE_ROW_BASE + len(dve_ops.OPS) - 1
    )
    return op


# h0p' = (h0^2 - 1) * t0
OP_SQM1_MUL = _register_op(
    "ANT_SQM1_MUL",
    (sq(Src0) - One) * Src1,
    lambda in0, in1: (in0 * in0 - 1.0) * in1,
)
# a0' = (h0^2 - 1) * (c_A0 - A0raw)
OP_A0G = _register_op(
    "ANT_A0G",
    (sq(Src0) - One) * (C0 - Src1),
    lambda in0, in1, s0: (in0 * in0 - 1.0) * (s0 - in1),
)
# e1 = h1 * (1 - h1^2) * t1^2
OP_E1G = _register_op(
    "ANT_E1G",
    Src0 * (One - sq(Src0)) * sq(Src1),
    lambda in0, in1: in0 * (1.0 - in0 * in0) * in1 * in1,
)


def build_nc():
    nc = bacc.Bacc()

    XTx = nc.dram_tensor("XTx", [DIM, BC], F16, kind="ExternalInput")
    XTv = nc.dram_tensor("XTv", [DIM, BC], F16, kind="ExternalInput")
    Fh = nc.dram_tensor("Fh", [128, NT * NCH * DIM], F16, kind="ExternalInput")
    # all small weights in two packed images (one DMA each)
    # pack16 cols: W1T[0:128] W1w[128:256] W08-block[256:384] W0g[384:392]
    #              w2m2[392] p2[393]
    Wp16 = nc.dram_tensor("Wp16", [128, 394], F16, kind="ExternalInput")
    Wp32 = nc.dram_tensor("Wp32", [128, 3], F32, kind="ExternalInput")
    out = nc.dram_tensor("out", [128, NT * NCH * DIM], F32, kind="ExternalOutput")

    with tile.TileContext(nc) as tc, ExitStack() as stk:
        consts = stk.enter_context(tc.tile_pool(name="consts", bufs=1))
        sb = stk.enter_context(tc.tile_pool(name="sb", bufs=2))
        ps_z0 = stk.enter_context(tc.tile_pool(name="psz0", bufs=2, space="PSUM"))
        ps_t0 = stk.enter_context(tc.tile_pool(name="pst0", bufs=2, space="PSUM"))
        ps_zA = stk.enter_context(tc.tile_pool(name="pszA", bufs=2, space="PSUM"))
        ps_t1 = stk.enter_context(tc.tile_pool(name="pst1", bufs=1, space="PSUM"))
        ps_bm = stk.enter_context(tc.tile_pool(name="psbm", bufs=1, space="PSUM"))

        # ---------------- constants / inputs ----------------
        XT64 = consts.tile([40, BC], F16)
        wp16 = consts.tile([128, 394], F16)
        wp32 = consts.tile([128, 3], F32)
        force_sb = consts.tile([128, NT * NCH * DIM], F16)
        nc.sync.dma_start(out=wp16, in_=Wp16[:, :])
        nc.sync.dma_start(out=wp32, in_=Wp32[:, :])
        # X transposed halves arrive in 4 column chunks so tile 0's matmuls
        # unblock after the first quarter of the transfer (subtile deps).
        XQ = BC // 4
        for q in range(4):
            qs = slice(q * XQ, (q + 1) * XQ)
            nc.sync.dma_start(out=XT64[0:DIM, qs], in_=XTx[:, qs])
            nc.sync.dma_start(out=XT64[32:40, qs], in_=XTv[:, qs])
        nc.gpsimd.dma_start(out=force_sb, in_=Fh[:, :])
        W1T_sb = wp16[:, 0:128]
        W1w_sb = wp16[:, 128:256]
        W08_sb = wp16[0:40, 256:384]
        W0g_sb = wp16[:, 384:392]
        w2m2_sb = wp16[:, 392:393]
        p2_sb = wp16[:, 393:394]
        b0_sb = wp32[:, 0:1]
        b1_sb = wp32[:, 1:2]
        cA0_sb = wp32[:, 2:3]

        out_sb = consts.tile([128, NT * NCH * DIM], F32)

        # HAM warmup: ~14 back-to-back dummy matmuls keep the PE busy for a
        # full activity window so the clock gate opens (1.2 -> 2.4 GHz) while
        # the input DMAs land; the main loop's PE gaps are all far below the
        # re-throttle window, so the array stays at full clock afterwards.
        wg = consts.tile([128, H], F16)
        wr = consts.tile([128, TW], F16)
        nc.gpsimd.memset(wg, 0.0)
        nc.gpsimd.memset(wr, 0.0)
        wps = ps_z0.tile([128, TW], F32, tag="z0")
        for _ in range(10):
            nc.tensor.matmul(wps, wg, wr, start=True, stop=True)

        st = [dict() for _ in range(NT)]
        bm_cur = [None]

        def ap3(t, off, dims):
            return bass.AP(
                tensor=t.tensor, offset=t.offset + off, ap=[list(t.ap[0])] + dims
            )

        # ---------------- software-pipelined main loop ----------------
        for i in range(NT + 3):
            j0, j1, j2, j3 = i, i - 1, i - 2, i - 3

            if 0 <= j0 < NT:
                s = st[j0]
                cols = slice(j0 * TW, (j0 + 1) * TW)
                z0 = ps_z0.tile([128, TW], F32, tag="z0")
                nc.tensor.matmul(
                    z0, W08_sb[0:DIM, :], XT64[0:DIM, cols], start=True, stop=True
                )
                t0 = ps_t0.tile([128, TW], F32, tag="t0")
                nc.tensor.matmul(
                    t0, W08_sb[32:40, :], XT64[32:40, cols], start=True, stop=True
                )
                s["z0"], s["t0"] = z0, t0

            if 0 <= j2 < NT:
                # e1's inputs (h1, t1 of tile j2) completed last iteration, so
                # it leads the DVE queue and fills the wait for this
                # iteration's h0 activation.
                s = st[j2]
                e1 = sb.tile([128, TW], F16, tag="e1")
                nc.vector._custom_dve(OP_E1G, out=e1, in0=s["h1"], in1=s["t1"][:, :])
                s["e1"] = e1
                A0 = ps_zA.tile([128, TW], F32, tag="zA")
                nc.tensor.matmul(A0, W1w_sb, s["q1"], start=True, stop=True)
                s["A0"] = A0

            if 0 <= j1 < NT:
                s = st[j1]
                h0 = sb.tile([128, TW], F16, tag="h0")
                nc.scalar.activation(h0, s["z0"], ACT.Tanh, bias=b0_sb[:, 0:1])
                t0sq = sb.tile([128, TW], F16, tag="t0sq")
                nc.scalar.activation(t0sq, s["t0"], ACT.Square)
                h0p = sb.tile([128, TW], F16, tag="h0p")
                nc.vector._custom_dve(OP_SQM1_MUL, out=h0p, in0=h0, in1=s["t0"][:, :])
                w0 = sb.tile([128, TW], F16, tag="w0")
                nc.gpsimd.tensor_mul(w0, h0, t0sq)
                z1 = ps_zA.tile([128, TW], F32, tag="zA")
                nc.tensor.matmul(z1, W1T_sb, h0, start=True, stop=True)
                h1 = sb.tile([128, TW], F16, tag="h1")
                nc.scalar.activation(h1, z1, ACT.Tanh, bias=b1_sb[:, 0:1])
                q1 = sb.tile([128, TW], F16, tag="q1")
                nc.scalar.activation(q1, h1, ACT.Square)
                s["h0"], s["h0p"], s["w0"], s["h1"], s["q1"] = h0, h0p, w0, h1, q1

            if 0 <= j2 < NT:
                s = st[j2]
                a0p = sb.tile([128, TW], F16, tag="a0p")
                nc.vector._custom_dve(
                    OP_A0G, out=a0p, in0=s["h0"], in1=s["A0"][:, :], s0=cA0_sb[:, 0:1]
                )
                e2 = sb.tile([128, TW], F16, tag="e2")
                nc.vector.tensor_mul(e2, a0p, s["w0"])
                s["a0p"], s["e2"] = a0p, e2

            if 0 <= j1 < NT:
                s = st[j1]
                t1 = ps_t1.tile([128, TW], F32, tag="t1")
                nc.tensor.matmul(t1, W1T_sb, s["h0p"], start=True, stop=True)
                s["t1"] = t1

            if 0 <= j3 < NT:
                # flipped per-chunk reductions for tile j3 (inputs from i-1);
                # late in the PE queue: nothing reads bm until the group's
                # bmc copy, so only z1/A0 need to run early for the DVE chain.
                s = st[j3]
                if j3 % GT == 0:
                    bm_cur[0] = ps_bm.tile(
                        [128, GT * NCH * CW], F32, tag="bm", name="bm"
                    )
                bm = bm_cur[0]
                base = (j3 % GT) * NCH * CW
                for c in range(NCH):
                    csl = slice(c * 128, (c + 1) * 128)
                    off = base + c * CW
                    nc.tensor.matmul(
                        bm[:, off : off + DIM],
                        s["a0p"][:, csl],
                        W0g_sb,
                        start=True,
                        stop=True,
                    )
                    nc.tensor.matmul(
                        bm[:, off + DIM : off + CW],
                        s["e1"][:, csl],
                        w2m2_sb,
                        start=True,
                        stop=False,
                    )
                    nc.tensor.matmul(
                        bm[:, off + DIM : off + CW],
                        s["e2"][:, csl],
                        p2_sb,
                        start=False,
                        stop=True,
                    )

            if 0 <= j3 < NT:
                bm = bm_cur[0]
                g = j3 // GT
                if j3 % GT == GT - 1:
                    # -------- batch-major tail for group g (GT*NCH chunks) --------
                    NCK = GT * NCH  # 16 chunks of 128 batch rows
                    bmc = sb.tile([128, NCK * CW], F32, tag="bmc")
                    nc.scalar.copy(bmc, bm[:, 0 : NCK * CW])
                    gv = ap3(bmc, 0, [[CW, NCK], [1, DIM]])
                    hv = ap3(bmc, DIM, [[CW, NCK]])
                    fv = ap3(force_sb, g * NCK * DIM, [[DIM, NCK], [1, DIM]])
                    prod = sb.tile([128, 2 * NCK * DIM], F32, tag="prod")
                    prod4 = prod.rearrange("p (q k f) -> p q k f", q=2, f=DIM)
                    nc.vector.tensor_mul(prod4[:, 0, :, :], gv, gv)
                    nc.vector.tensor_mul(prod4[:, 1, :, :], gv, fv)
                    red = sb.tile([128, 2 * NCK], F32, tag="red")
                    red3 = red.rearrange("p (q k) -> p q k", q=2)
                    nc.vector.tensor_reduce(red3, prod4, axis=AX.X, op=OP.add)
                    den = sb.tile([128, NCK], F32, tag="den")
                    nc.vector.tensor_scalar_add(den, red3[:, 0, :], 1.0)
                    rec = sb.tile([128, NCK], F32, tag="rec")
                    nc.vector.reciprocal(rec, den)
                    num = sb.tile([128, NCK], F32, tag="num")
                    nc.vector.tensor_sub(num, hv, red3[:, 1, :])
                    s2 = sb.tile([128, NCK], F32, tag="s2")
                    nc.vector.tensor_mul(s2, num, rec)
                    s2b = bass.AP(
                        tensor=s2.tensor,
                        offset=s2.offset,
                        ap=[list(s2.ap[0]), [1, NCK], [0, DIM]],
                    )
                    su = sb.tile([128, NCK * DIM], F32, tag="su")
                    su3 = su.rearrange("p (k f) -> p k f", f=DIM)
                    nc.vector.tensor_mul(su3, gv, s2b)
                    ov = ap3(out_sb, g * NCK * DIM, [[DIM, NCK], [1, DIM]])
                    nc.vector.tensor_add(ov, su3, fv)

        nc.sync.dma_start(out=out[:, :], in_=out_sb)

    if not nc.is_finalized():
        nc.finalize()

    return nc


_NC_CACHE = None


def _install_ntff_shim():
    """Register the axon NTFF profile hook (missing antenv.axon_hooks shim)."""
    import sys
    import types

    if "antenv.axon_hooks" in sys.modules:
        return
    try:
        sys.path.insert(0, "/root/.axon_site")
        from trn_agent_boot.trn_boot import _ntff_profile_via_ctypes

        hook = _ntff_profile_via_ctypes("/opt/axon/libaxon_pjrt.so")
        mod = types.ModuleType("antenv.axon_hooks")
        mod.get_axon_ntff_profile_hook = lambda: hook
        sys.modules["antenv.axon_hooks"] = mod
    except Exception:
        pass


def kernel(**inputs):
    global LAST_RESULTS, _NC_CACHE
    trace = bool(int(os.environ.get("KERNEL_TRACE", "0")))
    if trace:
        _install_ntff_shim()
    if _NC_CACHE is None:
        _NC_CACHE = build_nc()
    nc = _NC_CACHE

    f16 = np.float16
    X = np.asarray(inputs["X"], np.float32)
    K = np.asarray(inputs["K"], np.float32)
    D = np.asarray(inputs["D"], np.float32)
    W0 = np.asarray(inputs["W0"], np.float32)
    W1 = np.asarray(inputs["W1"], np.float32)
    w2 = np.asarray(inputs["W2"], np.float32).reshape(-1)

    w1w16 = (w2[:, None] * W1).astype(f16)
    ca0 = w1w16.astype(np.float32).sum(axis=0).astype(np.float32)

    wp16 = np.zeros((128, 394), f16)
    wp16[:, 0:128] = W1.T.astype(f16)
    wp16[0:DIM, 256:384] = W0.T.astype(f16)
    wp16[32:40, 256:384] = W0.T.astype(f16)
    wp16[:, 128:256] = w1w16
    wp16[:, 384:392] = W0.astype(f16)
    wp16[:, 392] = (-2.0 * w2).astype(f16)
    wp16[:, 393] = f16(2.0)
    wp32 = np.zeros((128, 3), np.float32)
    wp32[:, 0] = np.asarray(inputs["b0"], np.float32)
    wp32[:, 1] = np.asarray(inputs["b1"], np.float32)
    wp32[:, 2] = ca0

    KD = np.concatenate([K, D], axis=1)          # [8, 16]
    force = -(X @ KD.T)                           # [B, 8] f32 on host

    shared = {"Wp16": wp16, "Wp32": wp32}

    in_maps = []
    for i in range(NCORES):
        Xi = X[i * BC : (i + 1) * BC]
        Fi = force[i * BC : (i + 1) * BC]
        m = {
            "XTx": np.ascontiguousarray(Xi[:, :DIM].T).astype(f16),
            "XTv": np.ascontiguousarray(Xi[:, DIM:].T).astype(f16),
            "Fh": np.ascontiguousarray(
                Fi.reshape(NT, NCH, 128, DIM).transpose(2, 0, 1, 3).reshape(128, -1)
            ).astype(f16),
        }
        m.update(shared)
        in_maps.append(m)

    res = run_bass_kernel_spmd(
        nc, in_maps, core_ids=list(range(NCORES)), trace=trace
    )
    LAST_RESULTS = res
    outs = []
    for i in range(NCORES):
        o = res.results[i]["out"]  # [128, NT*NCH*DIM]
        o = o.reshape(128, NT, NCH, DIM).transpose(1, 2, 0, 3).reshape(BC, DIM)
        outs.append(o)
    return np.concatenate(outs, axis=0).astype(np.float32)
